# revision 12
# baseline (speedup 1.0000x reference)
"""Trainium2 Bass kernel for nn_CCA_Block (cross-channel attention block).

Reference computation (per batch element, B=8 sharded one-per-core):
    q = relu(x1 @ Wq); k = relu(x1 @ Wk); v = relu(x2 @ Wv)      # 1x1 convs
    scores[c,h,g] = scale * sum_w q[h,w,c] * k[g,w,c]
    attn = softmax(scores, axis=g)
    o[h,w,c] = sum_g attn[c,h,g] * v[g,w,c]
    g = sigmoid(o @ Ws + bs)
    g = gamma * (g - mu) / sqrt(var + eps) + beta
    out = x1 + x2 * g

Sharding: data-parallel over batch across the 8 NeuronCores (batch b -> core b).

Per-core dataflow (matmuls in bf16 with fp32 PSUM accumulate; transposes are
PE transpose-mode matmuls batched 4-wide into bf16 PSUM). The V-conv and
QK-conv pipelines are emitted interleaved to keep the PE array dense:
  V:  x2 w-major cast-DMA -> PE transpose -> x2T -> PE conv -> relu -> v_sb
  QK: x1 h-major cast-DMA -> PE transpose -> x1T -> fused q|k conv (N=256
      moving operand [Wq|Wk]) -> one relu evac -> qk_sb [w, h*256+qk*128+c]
  A:  per channel c: scoresT = kT_c' qT_c (PE) -> exp (ACT, scale folded)
      -> o_unnorm = E' V_c and Z = E' 1 (PE) -> 1/Z (DVE)
      -> o = o_unnorm * (1/Z bcast) + delta  (Ws^T delta = bs)
  G:  o_sb -> PE transpose -> conv with Ws -> sigmoid (ACT)
      -> BN affine (host-folded a,b) -> t = x2*g -> t += x1 (DMA accumulate)
"""

import numpy as np
import ml_dtypes

B, H, W, C = 8, 128, 128, 128
N_CORES = 8
BN_EPS = 1e-3

_BUILD_CACHE: dict = {}


def _build_program(scale_val: float, delta: tuple, bias_via_dve: bool, b_zero: bool):
    """Emit + compile the per-core Bass program. All cores run the identical
    program on their own batch slice."""
    import concourse.bacc as bacc
    import concourse.mybir as mybir
    import concourse.tile as tile

    fp32 = mybir.dt.float32
    bf16 = mybir.dt.bfloat16
    AF = mybir.ActivationFunctionType
    OP = mybir.AluOpType
    delta_zero = all(d == 0.0 for d in delta)

    nc = bacc.Bacc("TRN2", target_bir_lowering=False, debug=False,
                   enable_asserts=False)

    x1_d = nc.dram_tensor("x1", [H, W, C], fp32, kind="ExternalInput")
    x2_d = nc.dram_tensor("x2", [H, W, C], fp32, kind="ExternalInput")
    wqk_d = nc.dram_tensor("wqk", [C, 2 * C], bf16, kind="ExternalInput")
    wv_d = nc.dram_tensor("wv", [C, C], bf16, kind="ExternalInput")
    ws_d = nc.dram_tensor("ws", [C, C], bf16, kind="ExternalInput")
    ones_d = nc.dram_tensor("ones_col", [C, 1], bf16, kind="ExternalInput")
    ident_d = nc.dram_tensor("ident", [C, C], bf16, kind="ExternalInput")
    arep_d = nc.dram_tensor("a_rep", [C, 4 * C], bf16, kind="ExternalInput")
    brep_d = nc.dram_tensor("b_rep", [C, 4 * C], bf16, kind="ExternalInput")
    bsrep_d = nc.dram_tensor("bs_rep", [C, 4 * C], fp32, kind="ExternalInput")
    out_d = nc.dram_tensor("out", [H, W, C], fp32, kind="ExternalOutput")

    x1_ap, x2_ap, out_ap = x1_d.ap(), x2_d.ap(), out_d.ap()

    with tile.TileContext(nc) as tc:
        with (
            # persistent single-buffer pools
            tc.tile_pool(name="wts", bufs=1) as p_wts,
            tc.tile_pool(name="qkv", bufs=1) as p_qkv,
            tc.tile_pool(name="obuf", bufs=1) as p_o,
            # streaming pools
            tc.tile_pool(name="xcast", bufs=3) as p_xcast,
            tc.tile_pool(name="xT", bufs=6) as p_xT,
            tc.tile_pool(name="eexp", bufs=6) as p_e,
            tc.tile_pool(name="rz", bufs=6) as p_rz,
            tc.tile_pool(name="gres", bufs=4) as p_g,
            tc.tile_pool(name="x2f", bufs=4) as p_x2f,
            tc.tile_pool(name="outt", bufs=4) as p_out,
            # psum: shared full-bank fp32 tag (6) + bf16 transpose tag (2)
            tc.tile_pool(name="psA", bufs=6, space="PSUM") as ps_a,
            tc.tile_pool(name="psT", bufs=2, space="PSUM") as ps_t,
        ):
            # ---- constants ----
            wqk = p_wts.tile([C, 2 * C], bf16, tag="wqk")
            wv = p_wts.tile([C, C], bf16, tag="wv")
            ws = p_wts.tile([C, C], bf16, tag="ws")
            ones = p_wts.tile([C, 1], bf16, tag="ones")
            ident = p_wts.tile([C, C], bf16, tag="ident")
            arep = p_wts.tile([C, 4 * C], bf16, tag="arep")
            nc.sync.dma_start(wqk[:], wqk_d.ap())
            nc.sync.dma_start(wv[:], wv_d.ap())
            nc.sync.dma_start(ws[:], ws_d.ap())
            nc.sync.dma_start(ones[:], ones_d.ap())
            nc.sync.dma_start(ident[:], ident_d.ap())
            nc.sync.dma_start(arep[:], arep_d.ap())
            if not b_zero:
                brep = p_wts.tile([C, 4 * C], bf16, tag="brep")
                nc.sync.dma_start(brep[:], brep_d.ap())
            if bias_via_dve:
                bsrep = p_wts.tile([C, 4 * C], fp32, tag="bsrep")
                nc.sync.dma_start(bsrep[:], bsrep_d.ap())

            # persistent big buffers (bf16): free-axis layouts noted
            qk_sb = p_qkv.tile([W, H * 2 * C], bf16, tag="qk")  # [w,h*256+s*128+c]
            # v plus a trailing ones-column block: column W*C+c == 1.0 so a
            # single N=129 matmul computes both o_unnorm and the softmax
            # denominator Z (as output column 128)
            v_sb = p_qkv.tile([H, W * C + C], bf16, tag="v")    # [g, w*128+c]
            nc.vector.memset(v_sb[:, W * C :], 1.0)
            o_sb = p_o.tile([H, C * W], bf16, tag="o")          # [h, c*128+w]

            def transpose4(src_fn, evac_engine):
                """4 PE tile-transposes into one bf16 PSUM bank + wide evac.
                src_fn(j) -> [128,128] bf16 SBUF AP. Returns SBUF tile
                [128, 512] holding the 4 transposed tiles."""
                pst = ps_t.tile([C, 512], bf16, tag="pst")
                for j in range(4):
                    nc.tensor.matmul(
                        pst[:, j * C : (j + 1) * C], src_fn(j), ident[:],
                        is_transpose=True, start=(j == 0), stop=(j == 3),
                    )
                xt = p_xT.tile([C, 512], bf16, tag="xT")
                if evac_engine == "act":
                    nc.scalar.activation(xt[:], pst[:], AF.Copy)
                elif evac_engine == "dve":
                    nc.vector.tensor_copy(xt[:], pst[:])
                else:  # split halves across both engines in parallel
                    nc.scalar.activation(xt[:, :256], pst[:, :256], AF.Copy)
                    nc.vector.tensor_copy(xt[:, 256:], pst[:, 256:])
                return xt

            # ===== Phases V and QK, interleaved per 4-pixel group =====
            for p0 in range(0, W, 4):
                # --- V group: x2 -> x2T -> v ---
                xc2 = p_xcast.tile([H, 4 * C], bf16, tag="xc2")
                nc.gpsimd.dma_start(xc2[:], x2_ap[:, p0 : p0 + 4, :])
                x2T = transpose4(
                    lambda j: xc2[:, j * C : (j + 1) * C], "dve"
                )
                psv = ps_a.tile([H, 512], fp32, tag="ps")
                for j in range(4):
                    nc.tensor.matmul(
                        psv[:, j * C : (j + 1) * C],
                        x2T[:, j * C : (j + 1) * C], wv[:],
                        start=(j == 0), stop=(j == 3),
                    )
                nc.scalar.activation(
                    v_sb[:, p0 * C : (p0 + 4) * C], psv[:], AF.Relu
                )

                # --- QK group: x1 -> x1T -> fused q|k conv ---
                xc = p_xcast.tile([W, 4 * C], bf16, tag="xc")
                src = x1_ap[p0 : p0 + 4].rearrange("hh w c -> w hh c")
                nc.gpsimd.dma_start(xc[:], src)
                x1T = transpose4(
                    lambda j: xc[:, j * C : (j + 1) * C], "act"
                )
                for s in range(2):  # two 2-h conv subgroups
                    psqk = ps_a.tile([W, 512], fp32, tag="ps")
                    for t in range(2):
                        j = 2 * s + t
                        nc.tensor.matmul(
                            psqk[:, t * 256 : (t + 1) * 256],
                            x1T[:, j * C : (j + 1) * C], wqk[:],
                            start=(t == 0), stop=(t == 1),
                        )
                    h2 = p0 + 2 * s
                    if s == 0:
                        nc.vector.tensor_scalar(
                            qk_sb[:, h2 * 2 * C : (h2 + 2) * 2 * C],
                            psqk[:], 0.0, None, OP.max,
                        )
                    else:
                        nc.scalar.activation(
                            qk_sb[:, h2 * 2 * C : (h2 + 2) * 2 * C],
                            psqk[:], AF.Relu,
                        )

            # ============ Phase A: attention over channels ============
            # qk_sb free layout: h*256 + s*128 + c  (s=0 -> q, s=1 -> k)
            qk4 = qk_sb[:].rearrange("w (h s c) -> w h s c", s=2, c=C)
            groups = [(c0, min(3, C - c0)) for c0 in range(0, C, 3)]
            for c0, gs in groups:
                pss = ps_a.tile([H, gs * H], fp32, tag="ps")
                for j in range(gs):
                    c = c0 + j
                    nc.tensor.matmul(
                        pss[:, j * H : (j + 1) * H],
                        qk4[:, :, 1, c], qk4[:, :, 0, c],
                        start=(j == 0), stop=(j == gs - 1),
                    )
                e4 = p_e.tile([H, gs * H], bf16, tag="e4")
                for j in range(gs):
                    nc.scalar.activation(
                        e4[:, j * H : (j + 1) * H],
                        pss[:, j * H : (j + 1) * H], AF.Exp, scale=scale_val,
                    )
                pso = ps_a.tile([H, gs * 129], fp32, tag="ps")
                for j in range(gs):
                    c = c0 + j
                    nc.tensor.matmul(
                        pso[:, j * 129 : (j + 1) * 129],
                        e4[:, j * H : (j + 1) * H],
                        v_sb[:, c : c + W * C + 1 : C],
                        start=(j == 0), stop=(j == gs - 1),
                    )
                po = pso[:].rearrange("h (j x) -> h j x", x=129)
                rz = p_rz.tile([H, gs], fp32, tag="rz")
                nc.vector.reciprocal(rz[:], po[:, :, 128])
                if delta_zero:
                    # wide normalize: o = o_unnorm * (1/Z) with 1/Z
                    # broadcast along w via a stride-0 AP
                    rzb = rz[:].unsqueeze(2).broadcast_to([H, gs, C])
                    nc.vector.tensor_tensor(
                        o_sb[:, c0 * W : (c0 + gs) * W],
                        po[:, :, 0:128], rzb, OP.mult,
                    )
                else:
                    for j in range(gs):
                        c = c0 + j
                        dst = o_sb[:, c * W : (c + 1) * W]
                        src_ap = pso[:, j * 129 : j * 129 + 128]
                        if (c0 // 3) % 2 == 0:
                            nc.scalar.activation(
                                dst, src_ap, AF.Copy,
                                bias=float(delta[c]), scale=rz[:, j : j + 1],
                            )
                        else:
                            nc.vector.tensor_scalar(
                                dst, src_ap, rz[:, j : j + 1], float(delta[c]),
                                OP.mult, OP.add,
                            )

            # ============ Phase G: o -> oT -> conv -> sigmoid/BN/residual ====
            o3 = o_sb[:].rearrange("h (c w) -> h c w", w=W)
            for w0 in range(0, W, 4):
                # 4 transposes into one bf16 bank, then evacuate as TWO
                # independent half-tiles (ACT + DVE in parallel) so the
                # first conv matmuls only depend on the first half.
                pst = ps_t.tile([C, 512], bf16, tag="pst")
                for j in range(4):
                    nc.tensor.matmul(
                        pst[:, j * C : (j + 1) * C], o3[:, :, w0 + j],
                        ident[:], is_transpose=True,
                        start=(j == 0), stop=(j == 3),
                    )
                oTa = p_xT.tile([C, 256], bf16, tag="oTa")
                oTb = p_xT.tile([C, 256], bf16, tag="oTb")
                nc.scalar.activation(oTa[:], pst[:, :256], AF.Copy)
                nc.vector.tensor_copy(oTb[:], pst[:, 256:])
                psg = ps_a.tile([H, 512], fp32, tag="ps")
                for j in range(4):
                    half = oTa if j < 2 else oTb
                    nc.tensor.matmul(
                        psg[:, j * C : (j + 1) * C],
                        half[:, (j % 2) * H : (j % 2 + 1) * H], ws[:],
                        start=(j == 0), stop=(j == 3),
                    )
                if bias_via_dve:
                    nc.vector.tensor_tensor(psg[:], psg[:], bsrep[:], OP.add)
                g4 = p_g.tile([H, 512], bf16, tag="g4")
                nc.scalar.activation(g4[:], psg[:], AF.Sigmoid)
                gbn = p_g.tile([H, 512], bf16, tag="gbn")
                nc.vector.tensor_tensor(gbn[:], g4[:], arep[:], OP.mult)
                if not b_zero:
                    nc.vector.tensor_tensor(gbn[:], gbn[:], brep[:], OP.add)
                x2f = p_x2f.tile([H, 512], fp32, tag="x2f")
                nc.sync.dma_start(x2f[:], x2_ap[:, w0 : w0 + 4, :])
                t4 = p_out.tile([H, 512], fp32, tag="t4")
                if w0 % 8 == 0:
                    nc.vector.tensor_tensor(t4[:], x2f[:], gbn[:], OP.mult)
                else:
                    nc.gpsimd.tensor_tensor(t4[:], x2f[:], gbn[:], OP.mult)
                # residual add: t4 += x1 via SWDGE accumulate DMA
                nc.gpsimd.dma_start(
                    t4[:], x1_ap[:, w0 : w0 + 4, :], accum_op=OP.add
                )
                nc.sync.dma_start(out_ap[:, w0 : w0 + 4, :], t4[:])

    nc.compile()
    return nc


def _prepare(inputs):
    """Host-side prep: derived small tensors + baked scalars."""
    x1 = np.ascontiguousarray(np.asarray(inputs["x1"], dtype=np.float32))
    x2 = np.ascontiguousarray(np.asarray(inputs["x2"], dtype=np.float32))
    Wq = np.asarray(inputs["Wq"], dtype=np.float32)
    Wk = np.asarray(inputs["Wk"], dtype=np.float32)
    Wv = np.asarray(inputs["Wv"], dtype=np.float32)
    Ws = np.asarray(inputs["Ws"], dtype=np.float32)
    bs = np.asarray(inputs["bs"], dtype=np.float32)
    scale = float(np.asarray(inputs["scale"]).reshape(-1)[0])
    gamma = np.asarray(inputs["gamma"], dtype=np.float32)
    beta = np.asarray(inputs["beta"], dtype=np.float32)
    mu = np.asarray(inputs["mu"], dtype=np.float32)
    var = np.asarray(inputs["var"], dtype=np.float32)

    a = gamma / np.sqrt(var + BN_EPS)
    b = beta - mu * a
    b_zero = bool(np.all(b == 0.0))

    # fold the sigmoid bias bs into o:  o' = o + delta with Ws^T delta = bs
    bias_via_dve = False
    delta = np.zeros(C, dtype=np.float64)
    if np.any(bs != 0.0):
        try:
            delta = np.linalg.solve(Ws.astype(np.float64).T, bs.astype(np.float64))
            resid = np.abs(Ws.T @ delta.astype(np.float32) - bs).max()
            if not np.isfinite(delta).all() or resid > 1e-5 * (1 + np.abs(bs).max()):
                raise np.linalg.LinAlgError("bad solve")
        except np.linalg.LinAlgError:
            delta = np.zeros(C, dtype=np.float64)
            bias_via_dve = True

    bf = ml_dtypes.bfloat16
    consts = {
        "wqk": np.concatenate([Wq, Wk], axis=1).astype(bf),
        "wv": Wv.astype(bf),
        "ws": Ws.astype(bf),
        "ones_col": np.ones((C, 1), dtype=bf),
        "ident": np.eye(C, dtype=bf),
        "a_rep": np.tile(a, (C, 4)).astype(bf),
        "b_rep": np.tile(b, (C, 4)).astype(bf),
        "bs_rep": np.tile(bs, (C, 4)).astype(np.float32),
    }
    key = (scale, tuple(np.round(delta, 12)), bias_via_dve, b_zero)
    return x1, x2, consts, key, scale, delta, bias_via_dve, b_zero


def _get_nc(key, scale, delta, bias_via_dve, b_zero):
    if key not in _BUILD_CACHE:
        _BUILD_CACHE[key] = _build_program(scale, delta, bias_via_dve, b_zero)
    return _BUILD_CACHE[key]


def run(inputs, trace: bool = False):
    from concourse.bass_utils import run_bass_kernel_spmd

    x1, x2, consts, key, scale, delta, bias_via_dve, b_zero = _prepare(inputs)
    nc = _get_nc(key, scale, delta, bias_via_dve, b_zero)

    in_maps = []
    for core in range(N_CORES):
        m = dict(consts)
        m["x1"] = x1[core]
        m["x2"] = x2[core]
        in_maps.append(m)

    res = run_bass_kernel_spmd(
        nc, in_maps, core_ids=list(range(N_CORES)), trace=trace
    )
    out = np.stack([res.results[i]["out"] for i in range(N_CORES)], axis=0)
    return out.astype(np.float32), res


def kernel(**inputs) -> np.ndarray:
    out, _ = run(inputs, trace=False)
    return out


# revision 14
# speedup vs baseline: 1.0254x; 1.0254x over previous
"""Trainium2 Bass kernel for nn_CCA_Block (cross-channel attention block).

Reference computation (per batch element, B=8 sharded one-per-core):
    q = relu(x1 @ Wq); k = relu(x1 @ Wk); v = relu(x2 @ Wv)      # 1x1 convs
    scores[c,h,g] = scale * sum_w q[h,w,c] * k[g,w,c]
    attn = softmax(scores, axis=g)
    o[h,w,c] = sum_g attn[c,h,g] * v[g,w,c]
    g = sigmoid(o @ Ws + bs)
    g = gamma * (g - mu) / sqrt(var + eps) + beta
    out = x1 + x2 * g

Sharding: data-parallel over batch across the 8 NeuronCores (batch b -> core b).

Per-core dataflow (matmuls in bf16 with fp32 PSUM accumulate; transposes are
PE transpose-mode matmuls batched 4-wide into bf16 PSUM). The V-conv and
QK-conv pipelines are emitted interleaved to keep the PE array dense:
  V:  x2 w-major cast-DMA -> PE transpose -> x2T -> PE conv -> relu -> v_sb
  QK: x1 h-major cast-DMA -> PE transpose -> x1T -> fused q|k conv (N=256
      moving operand [Wq|Wk]) -> one relu evac -> qk_sb [w, h*256+qk*128+c]
  A:  per channel c: scoresT = kT_c' qT_c (PE) -> exp (ACT, scale folded)
      -> o_unnorm = E' V_c and Z = E' 1 (PE) -> 1/Z (DVE)
      -> o = o_unnorm * (1/Z bcast) + delta  (Ws^T delta = bs)
  G:  o_sb -> PE transpose -> conv with Ws -> sigmoid (ACT)
      -> BN affine (host-folded a,b) -> t = x2*g -> t += x1 (DMA accumulate)
"""

import numpy as np
import ml_dtypes

B, H, W, C = 8, 128, 128, 128
N_CORES = 8
BN_EPS = 1e-3

_BUILD_CACHE: dict = {}


def _build_program(scale_val: float, delta: tuple, bias_via_dve: bool, b_zero: bool):
    """Emit + compile the per-core Bass program. All cores run the identical
    program on their own batch slice."""
    import concourse.bacc as bacc
    import concourse.mybir as mybir
    import concourse.tile as tile

    fp32 = mybir.dt.float32
    bf16 = mybir.dt.bfloat16
    AF = mybir.ActivationFunctionType
    OP = mybir.AluOpType
    delta_zero = all(d == 0.0 for d in delta)

    nc = bacc.Bacc("TRN2", target_bir_lowering=False, debug=False,
                   enable_asserts=False)

    x1_d = nc.dram_tensor("x1", [H, W, C], fp32, kind="ExternalInput")
    x2_d = nc.dram_tensor("x2", [H, W, C], fp32, kind="ExternalInput")
    wqk_d = nc.dram_tensor("wqk", [C, 2 * C], bf16, kind="ExternalInput")
    wv_d = nc.dram_tensor("wv", [C, C], bf16, kind="ExternalInput")
    ws_d = nc.dram_tensor("ws", [C, C], bf16, kind="ExternalInput")
    ones_d = nc.dram_tensor("ones_col", [C, 1], bf16, kind="ExternalInput")
    ident_d = nc.dram_tensor("ident", [C, C], bf16, kind="ExternalInput")
    arep_d = nc.dram_tensor("a_rep", [C, 4 * C], bf16, kind="ExternalInput")
    brep_d = nc.dram_tensor("b_rep", [C, 4 * C], bf16, kind="ExternalInput")
    bsrep_d = nc.dram_tensor("bs_rep", [C, 4 * C], fp32, kind="ExternalInput")
    out_d = nc.dram_tensor("out", [H, W, C], fp32, kind="ExternalOutput")

    x1_ap, x2_ap, out_ap = x1_d.ap(), x2_d.ap(), out_d.ap()

    with tile.TileContext(nc) as tc:
        with (
            # persistent single-buffer pools
            tc.tile_pool(name="wts", bufs=1) as p_wts,
            tc.tile_pool(name="qkv", bufs=1) as p_qkv,
            tc.tile_pool(name="obuf", bufs=1) as p_o,
            # streaming pools
            tc.tile_pool(name="xcast", bufs=3) as p_xcast,
            tc.tile_pool(name="xT", bufs=6) as p_xT,
            tc.tile_pool(name="eexp", bufs=6) as p_e,
            tc.tile_pool(name="rz", bufs=6) as p_rz,
            tc.tile_pool(name="gres", bufs=4) as p_g,
            tc.tile_pool(name="x2f", bufs=4) as p_x2f,
            tc.tile_pool(name="outt", bufs=4) as p_out,
            # psum: shared full-bank fp32 tag (6) + bf16 transpose tag (2)
            tc.tile_pool(name="psA", bufs=5, space="PSUM") as ps_a,
            tc.tile_pool(name="psT", bufs=3, space="PSUM") as ps_t,
        ):
            # ---- constants ----
            wqk = p_wts.tile([C, 2 * C], bf16, tag="wqk")
            wv = p_wts.tile([C, C], bf16, tag="wv")
            ws = p_wts.tile([C, C], bf16, tag="ws")
            ones = p_wts.tile([C, 1], bf16, tag="ones")
            ident = p_wts.tile([C, C], bf16, tag="ident")
            arep = p_wts.tile([C, 4 * C], bf16, tag="arep")
            nc.sync.dma_start(wqk[:], wqk_d.ap())
            nc.sync.dma_start(wv[:], wv_d.ap())
            nc.sync.dma_start(ws[:], ws_d.ap())
            nc.sync.dma_start(ones[:], ones_d.ap())
            nc.sync.dma_start(ident[:], ident_d.ap())
            nc.sync.dma_start(arep[:], arep_d.ap())
            if not b_zero:
                brep = p_wts.tile([C, 4 * C], bf16, tag="brep")
                nc.sync.dma_start(brep[:], brep_d.ap())
            if bias_via_dve:
                bsrep = p_wts.tile([C, 4 * C], fp32, tag="bsrep")
                nc.sync.dma_start(bsrep[:], bsrep_d.ap())

            # persistent big buffers (bf16): free-axis layouts noted
            qk_sb = p_qkv.tile([W, H * 2 * C], bf16, tag="qk")  # [w,h*256+s*128+c]
            # v plus a trailing ones-column block: column W*C+c == 1.0 so a
            # single N=129 matmul computes both o_unnorm and the softmax
            # denominator Z (as output column 128)
            v_sb = p_qkv.tile([H, W * C + C], bf16, tag="v")    # [g, w*128+c]
            nc.vector.memset(v_sb[:, W * C :], 1.0)
            o_sb = p_o.tile([H, C * W], bf16, tag="o")          # [h, c*128+w]

            def transpose4(src_fn, evac_engine):
                """4 PE tile-transposes into one bf16 PSUM bank + wide evac.
                src_fn(j) -> [128,128] bf16 SBUF AP. Returns SBUF tile
                [128, 512] holding the 4 transposed tiles."""
                pst = ps_t.tile([C, 512], bf16, tag="pst")
                for j in range(4):
                    nc.tensor.matmul(
                        pst[:, j * C : (j + 1) * C], src_fn(j), ident[:],
                        is_transpose=True, start=(j == 0), stop=(j == 3),
                    )
                xt = p_xT.tile([C, 512], bf16, tag="xT")
                if evac_engine == "act":
                    nc.scalar.activation(xt[:], pst[:], AF.Copy)
                elif evac_engine == "dve":
                    nc.vector.tensor_copy(xt[:], pst[:])
                else:  # split halves across both engines in parallel
                    nc.scalar.activation(xt[:, :256], pst[:, :256], AF.Copy)
                    nc.vector.tensor_copy(xt[:, 256:], pst[:, 256:])
                return xt

            # ===== Phases V and QK, interleaved per 4-pixel group =====
            for p0 in range(0, W, 4):
                # --- V group: x2 -> x2T -> v ---
                xc2 = p_xcast.tile([H, 4 * C], bf16, tag="xc2")
                nc.gpsimd.dma_start(xc2[:], x2_ap[:, p0 : p0 + 4, :])
                x2T = transpose4(
                    lambda j: xc2[:, j * C : (j + 1) * C], "dve"
                )
                psv = ps_a.tile([H, 512], fp32, tag="ps")
                for j in range(4):
                    nc.tensor.matmul(
                        psv[:, j * C : (j + 1) * C],
                        x2T[:, j * C : (j + 1) * C], wv[:],
                        start=(j == 0), stop=(j == 3),
                    )
                nc.scalar.activation(
                    v_sb[:, p0 * C : (p0 + 4) * C], psv[:], AF.Relu
                )

                # --- QK group: x1 -> x1T -> fused q|k conv ---
                xc = p_xcast.tile([W, 4 * C], bf16, tag="xc")
                src = x1_ap[p0 : p0 + 4].rearrange("hh w c -> w hh c")
                nc.gpsimd.dma_start(xc[:], src)
                x1T = transpose4(
                    lambda j: xc[:, j * C : (j + 1) * C], "act"
                )
                for s in range(2):  # two 2-h conv subgroups
                    psqk = ps_a.tile([W, 512], fp32, tag="ps")
                    for t in range(2):
                        j = 2 * s + t
                        nc.tensor.matmul(
                            psqk[:, t * 256 : (t + 1) * 256],
                            x1T[:, j * C : (j + 1) * C], wqk[:],
                            start=(t == 0), stop=(t == 1),
                        )
                    h2 = p0 + 2 * s
                    if s == 0:
                        nc.vector.tensor_scalar(
                            qk_sb[:, h2 * 2 * C : (h2 + 2) * 2 * C],
                            psqk[:], 0.0, None, OP.max,
                        )
                    else:
                        nc.scalar.activation(
                            qk_sb[:, h2 * 2 * C : (h2 + 2) * 2 * C],
                            psqk[:], AF.Relu,
                        )

            # ============ Phase A: attention over channels ============
            # qk_sb free layout: h*256 + s*128 + c  (s=0 -> q, s=1 -> k)
            qk4 = qk_sb[:].rearrange("w (h s c) -> w h s c", s=2, c=C)
            groups = [(c0, min(3, C - c0)) for c0 in range(0, C, 3)]
            for c0, gs in groups:
                pss = ps_a.tile([H, gs * H], fp32, tag="ps")
                for j in range(gs):
                    c = c0 + j
                    nc.tensor.matmul(
                        pss[:, j * H : (j + 1) * H],
                        qk4[:, :, 1, c], qk4[:, :, 0, c],
                        start=(j == 0), stop=(j == gs - 1),
                    )
                e4 = p_e.tile([H, gs * H], bf16, tag="e4")
                for j in range(gs):
                    nc.scalar.activation(
                        e4[:, j * H : (j + 1) * H],
                        pss[:, j * H : (j + 1) * H], AF.Exp, scale=scale_val,
                    )
                pso = ps_a.tile([H, gs * 129], fp32, tag="ps")
                for j in range(gs):
                    c = c0 + j
                    nc.tensor.matmul(
                        pso[:, j * 129 : (j + 1) * 129],
                        e4[:, j * H : (j + 1) * H],
                        v_sb[:, c : c + W * C + 1 : C],
                        start=(j == 0), stop=(j == gs - 1),
                    )
                po = pso[:].rearrange("h (j x) -> h j x", x=129)
                rz = p_rz.tile([H, gs], fp32, tag="rz")
                nc.vector.reciprocal(rz[:], po[:, :, 128])
                if delta_zero:
                    # wide normalize: o = o_unnorm * (1/Z) with 1/Z
                    # broadcast along w via a stride-0 AP
                    rzb = rz[:].unsqueeze(2).broadcast_to([H, gs, C])
                    nc.vector.tensor_tensor(
                        o_sb[:, c0 * W : (c0 + gs) * W],
                        po[:, :, 0:128], rzb, OP.mult,
                    )
                else:
                    for j in range(gs):
                        c = c0 + j
                        dst = o_sb[:, c * W : (c + 1) * W]
                        src_ap = pso[:, j * 129 : j * 129 + 128]
                        if (c0 // 3) % 2 == 0:
                            nc.scalar.activation(
                                dst, src_ap, AF.Copy,
                                bias=float(delta[c]), scale=rz[:, j : j + 1],
                            )
                        else:
                            nc.vector.tensor_scalar(
                                dst, src_ap, rz[:, j : j + 1], float(delta[c]),
                                OP.mult, OP.add,
                            )

            # ============ Phase G: o -> oT -> conv -> sigmoid/BN/residual ====
            o3 = o_sb[:].rearrange("h (c w) -> h c w", w=W)
            for w0 in range(0, W, 4):
                oT = transpose4(lambda j: o3[:, :, w0 + j], "split")
                psg = ps_a.tile([H, 512], fp32, tag="ps")
                for j in range(4):
                    nc.tensor.matmul(
                        psg[:, j * C : (j + 1) * C],
                        oT[:, j * H : (j + 1) * H], ws[:],
                        start=(j == 0), stop=(j == 3),
                    )
                if bias_via_dve:
                    nc.vector.tensor_tensor(psg[:], psg[:], bsrep[:], OP.add)
                g4 = p_g.tile([H, 512], bf16, tag="g4")
                nc.scalar.activation(g4[:], psg[:], AF.Sigmoid)
                gbn = p_g.tile([H, 512], bf16, tag="gbn")
                nc.vector.tensor_tensor(gbn[:], g4[:], arep[:], OP.mult)
                if not b_zero:
                    nc.vector.tensor_tensor(gbn[:], gbn[:], brep[:], OP.add)
                x2f = p_x2f.tile([H, 512], fp32, tag="x2f")
                nc.sync.dma_start(x2f[:], x2_ap[:, w0 : w0 + 4, :])
                t4 = p_out.tile([H, 512], fp32, tag="t4")
                if w0 % 8 == 0:
                    nc.vector.tensor_tensor(t4[:], x2f[:], gbn[:], OP.mult)
                else:
                    nc.gpsimd.tensor_tensor(t4[:], x2f[:], gbn[:], OP.mult)
                # residual add: t4 += x1 via SWDGE accumulate DMA
                nc.gpsimd.dma_start(
                    t4[:], x1_ap[:, w0 : w0 + 4, :], accum_op=OP.add
                )
                nc.sync.dma_start(out_ap[:, w0 : w0 + 4, :], t4[:])

    nc.compile()
    return nc


def _prepare(inputs):
    """Host-side prep: derived small tensors + baked scalars."""
    x1 = np.ascontiguousarray(np.asarray(inputs["x1"], dtype=np.float32))
    x2 = np.ascontiguousarray(np.asarray(inputs["x2"], dtype=np.float32))
    Wq = np.asarray(inputs["Wq"], dtype=np.float32)
    Wk = np.asarray(inputs["Wk"], dtype=np.float32)
    Wv = np.asarray(inputs["Wv"], dtype=np.float32)
    Ws = np.asarray(inputs["Ws"], dtype=np.float32)
    bs = np.asarray(inputs["bs"], dtype=np.float32)
    scale = float(np.asarray(inputs["scale"]).reshape(-1)[0])
    gamma = np.asarray(inputs["gamma"], dtype=np.float32)
    beta = np.asarray(inputs["beta"], dtype=np.float32)
    mu = np.asarray(inputs["mu"], dtype=np.float32)
    var = np.asarray(inputs["var"], dtype=np.float32)

    a = gamma / np.sqrt(var + BN_EPS)
    b = beta - mu * a
    b_zero = bool(np.all(b == 0.0))

    # fold the sigmoid bias bs into o:  o' = o + delta with Ws^T delta = bs
    bias_via_dve = False
    delta = np.zeros(C, dtype=np.float64)
    if np.any(bs != 0.0):
        try:
            delta = np.linalg.solve(Ws.astype(np.float64).T, bs.astype(np.float64))
            resid = np.abs(Ws.T @ delta.astype(np.float32) - bs).max()
            if not np.isfinite(delta).all() or resid > 1e-5 * (1 + np.abs(bs).max()):
                raise np.linalg.LinAlgError("bad solve")
        except np.linalg.LinAlgError:
            delta = np.zeros(C, dtype=np.float64)
            bias_via_dve = True

    bf = ml_dtypes.bfloat16
    consts = {
        "wqk": np.concatenate([Wq, Wk], axis=1).astype(bf),
        "wv": Wv.astype(bf),
        "ws": Ws.astype(bf),
        "ones_col": np.ones((C, 1), dtype=bf),
        "ident": np.eye(C, dtype=bf),
        "a_rep": np.tile(a, (C, 4)).astype(bf),
        "b_rep": np.tile(b, (C, 4)).astype(bf),
        "bs_rep": np.tile(bs, (C, 4)).astype(np.float32),
    }
    key = (scale, tuple(np.round(delta, 12)), bias_via_dve, b_zero)
    return x1, x2, consts, key, scale, delta, bias_via_dve, b_zero


def _get_nc(key, scale, delta, bias_via_dve, b_zero):
    if key not in _BUILD_CACHE:
        _BUILD_CACHE[key] = _build_program(scale, delta, bias_via_dve, b_zero)
    return _BUILD_CACHE[key]


def run(inputs, trace: bool = False):
    from concourse.bass_utils import run_bass_kernel_spmd

    x1, x2, consts, key, scale, delta, bias_via_dve, b_zero = _prepare(inputs)
    nc = _get_nc(key, scale, delta, bias_via_dve, b_zero)

    in_maps = []
    for core in range(N_CORES):
        m = dict(consts)
        m["x1"] = x1[core]
        m["x2"] = x2[core]
        in_maps.append(m)

    res = run_bass_kernel_spmd(
        nc, in_maps, core_ids=list(range(N_CORES)), trace=trace
    )
    out = np.stack([res.results[i]["out"] for i in range(N_CORES)], axis=0)
    return out.astype(np.float32), res


def kernel(**inputs) -> np.ndarray:
    out, _ = run(inputs, trace=False)
    return out
